# revision 28
# baseline (speedup 1.0000x reference)
"""Multi-head causal attention on 8 TRN2 NeuronCores.

Sharding: core c -> (batch b = c//2, head-group g = c%2). Each core computes
Q/K/V projections for its 8 heads (512 of the 1024 channels), causal
attention, and the row-parallel W_o partial product; the host sums the two
partials per batch (the "all-reduce").

Device layouts (per core):
  xT   (1024, 2048) bf16   x[b] transposed (channels on partitions)
  wqT  (1024, 512)  bf16   W_q[rows g].T  -> lhsT for QT = Wq_g @ xT
  wkT  (1024, 512)  bf16   same for K
  wvT  (1024, 512)  bf16   rhs for natural-layout V = x @ Wv_g.T
  woT  (512, 1024)  bf16   W_o[:, cols g].T -> lhsT for yT = Wo_g @ O^T
  mask (128, 2048)  bf16   4 diagonal-block masks (128x512 each)
  yT   (1024, 2048) bf16   partial output, transposed (host sums in f32)

Attention per head h (d_k=64): scores are computed transposed,
S^T = K_h @ Q_h^T (k on partitions, q on free axis), exp on the scalar
engine (no max subtraction: |scores/8| < ~6 at these scales), multiplicative
0/1 mask on diagonal blocks only, and P^T is consumed directly as the moving
operand of out^T = [V_h | 1]^T @ P^T, whose row 64 accumulates the softmax
denominators Z. Diagonal-crossing blocks are computed only on their valid
column range.

The PE executes matmuls strictly serially (no row-group concurrency), so
wall clock ~= total PE stream cycles + stalls. The schedule therefore aims
at (a) DMA priority order so the PE starts early and never waits on weights,
(b) "filler" matmuls (deferred V tiles, later pairs' Q^T/K^T projections,
unlocked W_o chunks) woven into the attention S->exp->AV latency chain so
the PE never idles (idle gaps also re-throttle the PE clock 2.4->1.2 GHz),
(c) off-critical engines: upfront psum->sbuf casts and y staging copies on
the scalar engine's idle phases, mask/normalize on vector, broadcasts on
gpsimd.
"""

from collections import deque

import numpy as np

B, T, D = 4, 2048, 1024
NH, DK = 16, 64
NCORES = 8
HPC = NH // 2            # heads per core
HD = HPC * DK            # 512 head-dim channels per core
P = 128                  # partitions
NT = T // P              # 16 k-tiles
NQ = T // 512            # 4 q-blocks

_CACHE = {}


def _build():
    import concourse.mybir as mybir
    import concourse.tile as tile
    from concourse import bacc
    from concourse.tile import add_dep_helper

    f32, bf16 = mybir.dt.float32, mybir.dt.bfloat16
    Exp = mybir.ActivationFunctionType.Exp

    nc = bacc.Bacc(None, target_bir_lowering=False, debug=False)
    xT = nc.dram_tensor("xT", [D, T], bf16, kind="ExternalInput")
    wqT = nc.dram_tensor("wqT", [D, HD], bf16, kind="ExternalInput")
    wkT = nc.dram_tensor("wkT", [D, HD], bf16, kind="ExternalInput")
    wvT = nc.dram_tensor("wvT", [D, HD], bf16, kind="ExternalInput")
    woT = nc.dram_tensor("woT", [HD, D], bf16, kind="ExternalInput")
    mask = nc.dram_tensor("mask", [P, 4 * 1024], bf16, kind="ExternalInput")
    yT = nc.dram_tensor("yT", [D, T], bf16, kind="ExternalOutput")

    with tile.TileContext(nc) as tc:
        with (
            tc.tile_pool(name="persist", bufs=1) as persist,
            tc.tile_pool(name="work", bufs=6) as work,
            tc.tile_pool(name="psum", bufs=4, space="PSUM") as psum,
            tc.tile_pool(name="psum2", bufs=2, space="PSUM") as psum2,
        ):
            # ---- persistent tiles --------------------------------------
            xtc = [
                [persist.tile([P, 512], bf16, tag=f"x{c}_{t}", name=f"x{c}_{t}")
                 for t in range(NQ)]
                for c in range(8)
            ]
            wq_sb = persist.tile([P, 8, HD], bf16, tag="wq")
            wk_sb = persist.tile([P, 8, HD], bf16, tag="wk")
            wv_sb = persist.tile([P, 8, HD], bf16, tag="wv")
            wo_sb = persist.tile([P, 4, D], bf16, tag="wo")
            mask_sb = persist.tile([P, 4, 2, 512], bf16, tag="mask")
            qt = [persist.tile([P, T], bf16, tag=f"qt{a}", name=f"qt{a}")
                  for a in range(4)]
            # K^T stored zero-padded per head: ktz[a][hh] has head hh's 64
            # d_k rows in their partition range and ZEROS in the other 64,
            # so S matmuls use a full K=128 lhsT (enables FWL, which hides
            # LDWEIGHTS; K=64 matmuls pay ~100ns exposed load each).
            ktz = [[persist.tile([P, T], bf16, tag=f"kt{a}_{hh}",
                                 name=f"kt{a}_{hh}")
                    for hh in (0, 1)]
                   for a in range(4)]
            vt = [persist.tile([P, HPC, DK + 1], bf16, tag=f"v{tt}", name=f"v{tt}")
                  for tt in range(NT)]
            otn = [persist.tile([P, T], bf16, tag=f"otn{i}", name=f"otn{i}")
                   for i in range(4)]

            # ---- input DMAs -------------------------------------------
            # DMA pages round-robin across all queues, so anything enqueued
            # early steals bandwidth from everything else. Enforce priority
            # WAVES with deps (a dep delays the enqueue): wave0 = wv + x
            # tch0 (first V matmuls), wave1 = wq/wk + x tch1 (pair-0
            # projections), wave2 = mask + x tch2/3 (rest of upfront),
            # wave3 = wo (needed only at pair 3).
            nc.sync.dma_start(out=wv_sb, in_=wvT.rearrange("(co p) d -> p co d", p=P))
            xT_r = xT.rearrange("(co p) t -> co p t", p=P)
            xdma = {}
            wdma = {}

            def wdma_start(nm, dst, srcp, gates):
                if srcp is None:
                    ins = nc.sync.dma_start(
                        out=mask_sb,
                        in_=mask.rearrange("p (r g q) -> p r g q", g=2, q=512),
                    )
                else:
                    ins = nc.sync.dma_start(
                        out=dst, in_=srcp.rearrange("(co p) d -> p co d", p=P)
                    )
                for g in gates:
                    add_dep_helper(ins.ins, g, sync=True,
                                   reason="DMA priority wave")
                wdma[nm] = ins.ins

            for c in range(8):
                ins = nc.sync.dma_start(out=xtc[c][0], in_=xT_r[c][:, 0:512])
                if c >= 4:
                    # second half-wave: lets x[0..3] land early so c-major
                    # V matmuls start on partial data
                    add_dep_helper(ins.ins, xdma[3, 0], sync=True,
                                   reason="DMA priority wave")
                xdma[c, 0] = ins.ins
            wdma_start("wq", wq_sb, wqT, [xdma[3, 0]])
            wdma_start("wk", wk_sb, wkT, [wdma["wq"]])
            for c in range(8):
                ins = nc.sync.dma_start(out=xtc[c][1],
                                        in_=xT_r[c][:, 512:1024])
                add_dep_helper(ins.ins, xdma[c, 0], sync=True,
                               reason="DMA priority wave")
                add_dep_helper(ins.ins, wdma["wk"], sync=True,
                               reason="DMA priority wave")
                xdma[c, 1] = ins.ins
            wdma_start("mask", mask_sb, None, [wdma["wk"]])
            for tch in (2, 3):
                for c in range(8):
                    ins = nc.sync.dma_start(
                        out=xtc[c][tch],
                        in_=xT_r[c][:, 512 * tch:512 * tch + 512],
                    )
                    add_dep_helper(ins.ins, xdma[c, tch - 1], sync=True,
                                   reason="DMA priority wave")
                    if tch == 2:
                        add_dep_helper(ins.ins, wdma["wq"], sync=True,
                                       reason="DMA priority wave")
                    xdma[c, tch] = ins.ins
            wdma_start("wo", wo_sb, woT, [wdma["mask"]])
            for tt in range(NT):
                nc.vector.memset(vt[tt][:, :, DK:DK + 1], 1.0)
            for a in range(4):
                nc.vector.memset(ktz[a][0][64:128, :], 0.0)
                nc.vector.memset(ktz[a][1][0:64, :], 0.0)

            # ---- op builders (each closure emits one PE matmul) --------
            def v_tile_ops(tt):
                st = {}

                def mk(c):
                    def op():
                        if c == 0:
                            st["ps"] = psum.tile([P, HD], f32, tag="ps",
                                                 name=f"vps{tt}")
                        nc.tensor.matmul(
                            st["ps"],
                            lhsT=xtc[c][tt // 4][:, P * (tt % 4):P * (tt % 4) + P],
                            rhs=wv_sb[:, c, :],
                            start=(c == 0),
                            stop=(c == 7),
                        )
                        if c == 7:
                            nc.scalar.copy(
                                vt[tt][:, :, 0:DK],
                                st["ps"].rearrange("p (h e) -> p h e", e=DK),
                            )
                    return op

                return [mk(c) for c in range(8)]

            def proj_tile_ops(nm, w_sb, a, tch, cast):
                st = {}

                def mk(c):
                    def op():
                        if c == 0:
                            st["ps"] = psum.tile([P, 512], f32, tag="ps",
                                                 name=f"{nm}ps{a}_{tch}")
                        nc.tensor.matmul(
                            st["ps"],
                            lhsT=w_sb[:, c, 128 * a:128 * a + 128],
                            rhs=xtc[c][tch],
                            start=(c == 0),
                            stop=(c == 7),
                        )
                        if c == 7:
                            cast(st["ps"])
                    return op

                return [mk(c) for c in range(8)]

            def qt_cast(a, tch, on_scalar):
                def cast(ps):
                    dst = qt[a][:, 512 * tch:512 * tch + 512]
                    if on_scalar:
                        nc.scalar.copy(dst, ps)
                    else:
                        nc.vector.tensor_copy(dst, ps)
                return cast

            def kt_cast(a, tch, on_scalar):
                def cast(ps):
                    sl = slice(512 * tch, 512 * tch + 512)
                    for hh in (0, 1):
                        rows = slice(64 * hh, 64 * hh + 64)
                        dst = ktz[a][hh][rows, sl]
                        if on_scalar:
                            nc.scalar.copy(dst, ps[rows, :])
                        else:
                            nc.vector.tensor_copy(dst, ps[rows, :])
                return cast

            def wo_tile_ops(dt_, tch):
                st = {}

                def mk(c):
                    def op():
                        if c == 0:
                            st["ps"] = psum.tile([P, 512], f32, tag="ps",
                                                 name=f"yps{dt_}_{tch}")
                        nc.tensor.matmul(
                            st["ps"],
                            lhsT=wo_sb[:, c, 128 * dt_:128 * dt_ + 128],
                            rhs=otn[c][:, 512 * tch:512 * tch + 512],
                            start=(c == 0),
                            stop=(c == 3),
                        )
                        if c == 3:
                            yst = work.tile([P, 512], bf16, tag="yst", bufs=3,
                                            name=f"yst{dt_}_{tch}")
                            nc.scalar.copy(yst, st["ps"])
                            nc.sync.dma_start(
                                out=yT[128 * dt_:128 * dt_ + 128,
                                       512 * tch:512 * tch + 512],
                                in_=yst,
                            )
                    return op

                return [mk(c) for c in range(4)]

            # Minimal upfront work (everything else becomes filler): V tiles
            # for x tch0/1, and pair 0's full Q^T/K^T. Casts on scalar (idle
            # here).
            # V(tch0) c-major: 4 psum chains open at once, so the c<4
            # matmuls run as soon as the first half-wave of x lands
            v0_ops = [v_tile_ops(tt) for tt in range(4)]
            for c in range(8):
                for ops in v0_ops:
                    ops[c]()
            for tch in range(2):
                if tch == 1:
                    for tt in range(4, 8):
                        for op in v_tile_ops(tt):
                            op()
                for op in proj_tile_ops("qt", wq_sb, 0, tch,
                                        qt_cast(0, tch, True)):
                    op()
                for op in proj_tile_ops("kt", wk_sb, 0, tch,
                                        kt_cast(0, tch, True)):
                    op()
            for tch in range(2, NQ):
                for op in proj_tile_ops("qt", wq_sb, 0, tch,
                                        qt_cast(0, tch, True)):
                    op()
                for op in proj_tile_ops("kt", wk_sb, 0, tch,
                                        kt_cast(0, tch, True)):
                    op()

            # filler queue: deferred V tiles (needed from (0,2) on), then
            # later pairs' projections; W_o chunks are appended as each
            # (3, j) block unlocks them.
            fillers = deque()
            for tt in range(8, NT):
                fillers.extend(v_tile_ops(tt))
            for a in range(1, 4):
                for tch in range(NQ):
                    fillers.extend(proj_tile_ops("qt", wq_sb, a, tch,
                                                 qt_cast(a, tch, False)))
                    fillers.extend(proj_tile_ops("kt", wk_sb, a, tch,
                                                 kt_cast(a, tch, False)))

            def pull(n):
                for _ in range(n):
                    if fillers:
                        fillers.popleft()()

            # ---- attention: head pairs, software-pipelined -------------
            # Block = (head pair a, q-block j). Both heads' scores land in
            # one 2-bank PSUM tile so a single strided exp covers them.
            # Pair 3 runs j = 1,2,3,0 so each (3,j) unlocks W_o[*,j] while
            # later blocks still provide PE cover; the short (3,0) block
            # drains W_o[*,3] at a high fill rate and only W_o[*,0] trails.
            # A block's normalize chain (z->recip->bcast->mul) is deferred
            # into the NEXT block's step loop so the vector queue serves the
            # next block's mask muls first (the AV path) and the PE keeps
            # streaming while the chain runs.
            pull_rate = {0: 3, 1: 2, 2: 2, 3: 3}
            pending_norm = []
            for a in range(4):
                for j in range(NQ):
                    av = {
                        hh: psum.tile([DK + 1, 512], f32, tag="ps",
                                      name=f"av{a}_{hh}_{j}")
                        for hh in (0, 1)
                    }
                    n_k = 4 * j + 4
                    s_tiles = {}

                    def emit_s(k0, a=a, j=j, s_tiles=s_tiles):
                        r = k0 - 4 * j
                        lo = 128 * r if r > 0 else 0
                        sp = psum2.tile([P, 2, 512], f32, tag="s2",
                                        name=f"sps{a}_{j}")
                        s_tiles[k0] = (sp, lo, r)
                        for hh in (0, 1):
                            nc.tensor.matmul(
                                sp[:, hh, lo:512],
                                lhsT=ktz[a][hh][:, P * k0:P * k0 + P],
                                rhs=qt[a][:, 512 * j + lo:512 * j + 512],
                                start=True,
                                stop=True,
                            )

                    # S runs two steps ahead of AV so the in-order PE never
                    # blocks on the exp chain or this block's av slot alloc.
                    emit_s(0)
                    emit_s(1)
                    for k0 in range(n_k):
                        s_ps, lo, r = s_tiles.pop(k0)
                        u_t = work.tile([P, 2, 512], bf16, tag="u", bufs=10,
                                        name=f"u{a}_{j}")
                        nc.scalar.activation(
                            u_t[:, :, lo:512], s_ps[:, :, lo:512], Exp,
                            scale=0.125,
                        )
                        if r >= 0:
                            nc.vector.tensor_mul(
                                u_t[:, :, lo:512],
                                u_t[:, :, lo:512],
                                mask_sb[:, r, :, lo:512],
                            )
                        rate = 1 if (a == 3 and j == 3) else pull_rate[a]
                        pull(rate + (1 if r >= 1 else 0))
                        if k0 == 2 and pending_norm:
                            pending_norm.pop(0)()
                        if k0 + 2 < n_k:
                            emit_s(k0 + 2)
                        for hh in (0, 1):
                            nc.tensor.matmul(
                                av[hh][:, lo:512],
                                lhsT=vt[k0][:, 2 * a + hh, :],
                                rhs=u_t[:, hh, lo:512],
                                start=(k0 == 0),
                                stop=(k0 == n_k - 1),
                            )
                    # ---- normalize: otn = av[:64] * bcast(1/Z) --------
                    def normalize(a=a, j=j, av=av):
                        for hh in (0, 1):
                            poff = 64 * hh
                            z_sb = work.tile([1, 512], f32, tag="z", bufs=2,
                                             name=f"z{a}_{j}")
                            nc.vector.tensor_copy(z_sb, av[hh][DK:DK + 1, :])
                            rz = work.tile([1, 512], f32, tag="rz", bufs=2,
                                           name=f"rz{a}_{j}")
                            nc.vector.reciprocal_approx_fast(rz, z_sb)
                            bc = work.tile([64, 512], f32, tag="bc", bufs=2,
                                           name=f"bc{a}_{j}")
                            nc.gpsimd.partition_broadcast(bc, rz)
                            nc.vector.tensor_mul(
                                otn[a][poff:poff + 64, 512 * j:512 * j + 512],
                                av[hh][0:DK, :],
                                bc,
                            )
                        if a == 3:
                            for dt_ in range(8):
                                fillers.extend(wo_tile_ops(dt_, j))

                    if a < 3 and j == 3:
                        # defer pair-boundary normalize into (a+1, 0)'s step
                        # loop: the vector queue serves the next pair's mask
                        # muls (AV critical path) before this chain
                        pending_norm.append(normalize)
                    else:
                        if a == 3 and j == 3:
                            # reserved fillers bridge the last normalize
                            # chain before W_o[*,3] unlocks
                            pull(16)
                        normalize()

            # ---- drain: whatever W_o work wasn't woven in --------------
            pull(len(fillers))

    nc.finalize()
    return nc


def _get_nc():
    if "nc" not in _CACHE:
        _CACHE["nc"] = _build()
    return _CACHE["nc"]


def kernel(x, W_q, W_k, W_v, W_o):
    import ml_dtypes
    from concourse.bass_utils import run_bass_kernel_spmd

    bf16 = ml_dtypes.bfloat16
    x = np.asarray(x, dtype=np.float32)
    W_q = np.asarray(W_q, dtype=np.float32)
    W_k = np.asarray(W_k, dtype=np.float32)
    W_v = np.asarray(W_v, dtype=np.float32)
    W_o = np.asarray(W_o, dtype=np.float32)

    kk = np.arange(P)[:, None]
    qq = np.arange(512)[None, :]
    mask = np.concatenate(
        [np.tile(qq >= kk + 128 * r, (1, 2)) for r in range(4)], axis=1
    ).astype(bf16)

    in_maps = []
    for c in range(NCORES):
        b, g = c // 2, c % 2
        rows = slice(HD * g, HD * g + HD)
        in_maps.append(
            {
                "xT": np.ascontiguousarray(x[b].T).astype(bf16),
                "wqT": np.ascontiguousarray(W_q[rows, :].T).astype(bf16),
                "wkT": np.ascontiguousarray(W_k[rows, :].T).astype(bf16),
                "wvT": np.ascontiguousarray(W_v[rows, :].T).astype(bf16),
                "woT": np.ascontiguousarray(W_o[:, rows].T).astype(bf16),
                "mask": mask,
            }
        )

    res = run_bass_kernel_spmd(_get_nc(), in_maps, list(range(NCORES)))
    y = np.zeros((B, T, D), np.float32)
    for c in range(NCORES):
        y[c // 2] += res.results[c]["yT"].T.astype(np.float32)
    return y


# revision 29
# speedup vs baseline: 1.0174x; 1.0174x over previous
"""Multi-head causal attention on 8 TRN2 NeuronCores.

Sharding: core c -> (batch b = c//2, head-group g = c%2). Each core computes
Q/K/V projections for its 8 heads (512 of the 1024 channels), causal
attention, and the row-parallel W_o partial product; the host sums the two
partials per batch (the "all-reduce").

Device layouts (per core):
  xT   (1024, 2048) bf16   x[b] transposed (channels on partitions)
  wqT  (1024, 512)  bf16   W_q[rows g].T  -> lhsT for QT = Wq_g @ xT
  wkT  (1024, 512)  bf16   same for K
  wvT  (1024, 512)  bf16   rhs for natural-layout V = x @ Wv_g.T
  woT  (512, 1024)  bf16   W_o[:, cols g].T -> lhsT for yT = Wo_g @ O^T
  mask (128, 2048)  bf16   4 diagonal-block masks (128x512 each)
  yT   (1024, 2048) bf16   partial output, transposed (host sums in f32)

Attention per head h (d_k=64): scores are computed transposed,
S^T = K_h @ Q_h^T (k on partitions, q on free axis), exp on the scalar
engine (no max subtraction: |scores/8| < ~6 at these scales), multiplicative
0/1 mask on diagonal blocks only, and P^T is consumed directly as the moving
operand of out^T = [V_h | 1]^T @ P^T, whose row 64 accumulates the softmax
denominators Z. Diagonal-crossing blocks are computed only on their valid
column range.

The PE executes matmuls strictly serially (no row-group concurrency), so
wall clock ~= total PE stream cycles + stalls. The schedule therefore aims
at (a) DMA priority order so the PE starts early and never waits on weights,
(b) "filler" matmuls (deferred V tiles, later pairs' Q^T/K^T projections,
unlocked W_o chunks) woven into the attention S->exp->AV latency chain so
the PE never idles (idle gaps also re-throttle the PE clock 2.4->1.2 GHz),
(c) off-critical engines: upfront psum->sbuf casts and y staging copies on
the scalar engine's idle phases, mask/normalize on vector, broadcasts on
gpsimd.
"""

from collections import deque

import numpy as np

B, T, D = 4, 2048, 1024
NH, DK = 16, 64
NCORES = 8
HPC = NH // 2            # heads per core
HD = HPC * DK            # 512 head-dim channels per core
P = 128                  # partitions
NT = T // P              # 16 k-tiles
NQ = T // 512            # 4 q-blocks

_CACHE = {}


def _build():
    import concourse.mybir as mybir
    import concourse.tile as tile
    from concourse import bacc
    from concourse.tile import add_dep_helper

    f32, bf16 = mybir.dt.float32, mybir.dt.bfloat16
    Exp = mybir.ActivationFunctionType.Exp

    nc = bacc.Bacc(None, target_bir_lowering=False, debug=False)
    xT = nc.dram_tensor("xT", [D, T], bf16, kind="ExternalInput")
    wqT = nc.dram_tensor("wqT", [D, HD], bf16, kind="ExternalInput")
    wkT = nc.dram_tensor("wkT", [D, HD], bf16, kind="ExternalInput")
    wvT = nc.dram_tensor("wvT", [D, HD], bf16, kind="ExternalInput")
    woT = nc.dram_tensor("woT", [HD, D], bf16, kind="ExternalInput")
    mask = nc.dram_tensor("mask", [P, 4 * 1024], bf16, kind="ExternalInput")
    yT = nc.dram_tensor("yT", [D, T], bf16, kind="ExternalOutput")

    with tile.TileContext(nc) as tc:
        with (
            tc.tile_pool(name="persist", bufs=1) as persist,
            tc.tile_pool(name="work", bufs=6) as work,
            tc.tile_pool(name="psum", bufs=4, space="PSUM") as psum,
            tc.tile_pool(name="psum2", bufs=2, space="PSUM") as psum2,
        ):
            # ---- persistent tiles --------------------------------------
            xtc = [
                [persist.tile([P, 512], bf16, tag=f"x{c}_{t}", name=f"x{c}_{t}")
                 for t in range(NQ)]
                for c in range(8)
            ]
            wq_sb = persist.tile([P, 8, HD], bf16, tag="wq")
            wk_sb = persist.tile([P, 8, HD], bf16, tag="wk")
            wv_sb = persist.tile([P, 8, HD], bf16, tag="wv")
            wo_sb = persist.tile([P, 4, D], bf16, tag="wo")
            mask_sb = persist.tile([P, 4, 2, 512], bf16, tag="mask")
            qt = [persist.tile([P, T], bf16, tag=f"qt{a}", name=f"qt{a}")
                  for a in range(4)]
            # K^T stored zero-padded per head: ktz[a][hh] has head hh's 64
            # d_k rows in their partition range and ZEROS in the other 64,
            # so S matmuls use a full K=128 lhsT (enables FWL, which hides
            # LDWEIGHTS; K=64 matmuls pay ~100ns exposed load each).
            ktz = [[persist.tile([P, T], bf16, tag=f"kt{a}_{hh}",
                                 name=f"kt{a}_{hh}")
                    for hh in (0, 1)]
                   for a in range(4)]
            vt = [persist.tile([P, HPC, DK + 1], bf16, tag=f"v{tt}", name=f"v{tt}")
                  for tt in range(NT)]
            otn = [persist.tile([P, T], bf16, tag=f"otn{i}", name=f"otn{i}")
                   for i in range(4)]

            # ---- input DMAs -------------------------------------------
            # DMA pages round-robin across all queues, so anything enqueued
            # early steals bandwidth from everything else. Enforce priority
            # WAVES with deps (a dep delays the enqueue): wave0 = wv + x
            # tch0 (first V matmuls), wave1 = wq/wk + x tch1 (pair-0
            # projections), wave2 = mask + x tch2/3 (rest of upfront),
            # wave3 = wo (needed only at pair 3).
            nc.sync.dma_start(out=wv_sb, in_=wvT.rearrange("(co p) d -> p co d", p=P))
            xT_r = xT.rearrange("(co p) t -> co p t", p=P)
            xdma = {}
            wdma = {}

            def wdma_start(nm, dst, srcp, gates):
                if srcp is None:
                    ins = nc.sync.dma_start(
                        out=mask_sb,
                        in_=mask.rearrange("p (r g q) -> p r g q", g=2, q=512),
                    )
                else:
                    ins = nc.sync.dma_start(
                        out=dst, in_=srcp.rearrange("(co p) d -> p co d", p=P)
                    )
                for g in gates:
                    add_dep_helper(ins.ins, g, sync=True,
                                   reason="DMA priority wave")
                wdma[nm] = ins.ins

            for c in range(8):
                ins = nc.sync.dma_start(out=xtc[c][0], in_=xT_r[c][:, 0:512])
                if c >= 4:
                    # second half-wave: lets x[0..3] land early so c-major
                    # V matmuls start on partial data
                    add_dep_helper(ins.ins, xdma[3, 0], sync=True,
                                   reason="DMA priority wave")
                xdma[c, 0] = ins.ins
            wdma_start("wq", wq_sb, wqT, [xdma[3, 0]])
            wdma_start("wk", wk_sb, wkT, [wdma["wq"]])
            for c in range(8):
                ins = nc.sync.dma_start(out=xtc[c][1],
                                        in_=xT_r[c][:, 512:1024])
                add_dep_helper(ins.ins, xdma[c, 0], sync=True,
                               reason="DMA priority wave")
                add_dep_helper(ins.ins, wdma["wk"], sync=True,
                               reason="DMA priority wave")
                xdma[c, 1] = ins.ins
            wdma_start("mask", mask_sb, None, [wdma["wk"]])
            for tch in (2, 3):
                for c in range(8):
                    ins = nc.sync.dma_start(
                        out=xtc[c][tch],
                        in_=xT_r[c][:, 512 * tch:512 * tch + 512],
                    )
                    add_dep_helper(ins.ins, xdma[c, tch - 1], sync=True,
                                   reason="DMA priority wave")
                    if tch == 2:
                        add_dep_helper(ins.ins, wdma["wq"], sync=True,
                                       reason="DMA priority wave")
                    xdma[c, tch] = ins.ins
            wdma_start("wo", wo_sb, woT, [wdma["mask"]])
            for tt in range(NT):
                nc.vector.memset(vt[tt][:, :, DK:DK + 1], 1.0)
            for a in range(4):
                nc.vector.memset(ktz[a][0][64:128, :], 0.0)
                nc.vector.memset(ktz[a][1][0:64, :], 0.0)

            # ---- op builders (each closure emits one PE matmul) --------
            def v_tile_ops(tt):
                st = {}

                def mk(c):
                    def op():
                        if c == 0:
                            st["ps"] = psum.tile([P, HD], f32, tag="ps",
                                                 name=f"vps{tt}")
                        nc.tensor.matmul(
                            st["ps"],
                            lhsT=xtc[c][tt // 4][:, P * (tt % 4):P * (tt % 4) + P],
                            rhs=wv_sb[:, c, :],
                            start=(c == 0),
                            stop=(c == 7),
                        )
                        if c == 7:
                            nc.scalar.copy(
                                vt[tt][:, :, 0:DK],
                                st["ps"].rearrange("p (h e) -> p h e", e=DK),
                            )
                    return op

                return [mk(c) for c in range(8)]

            def proj_tile_ops(nm, w_sb, a, tch, cast):
                st = {}

                def mk(c):
                    def op():
                        if c == 0:
                            st["ps"] = psum.tile([P, 512], f32, tag="ps",
                                                 name=f"{nm}ps{a}_{tch}")
                        nc.tensor.matmul(
                            st["ps"],
                            lhsT=w_sb[:, c, 128 * a:128 * a + 128],
                            rhs=xtc[c][tch],
                            start=(c == 0),
                            stop=(c == 7),
                        )
                        if c == 7:
                            cast(st["ps"])
                    return op

                return [mk(c) for c in range(8)]

            def qt_cast(a, tch, on_scalar):
                def cast(ps):
                    dst = qt[a][:, 512 * tch:512 * tch + 512]
                    if on_scalar:
                        nc.scalar.copy(dst, ps)
                    else:
                        nc.vector.tensor_copy(dst, ps)
                return cast

            def kt_cast(a, tch, on_scalar):
                def cast(ps):
                    sl = slice(512 * tch, 512 * tch + 512)
                    for hh in (0, 1):
                        rows = slice(64 * hh, 64 * hh + 64)
                        dst = ktz[a][hh][rows, sl]
                        if on_scalar:
                            nc.scalar.copy(dst, ps[rows, :])
                        else:
                            nc.vector.tensor_copy(dst, ps[rows, :])
                return cast

            def wo_tile_ops(dt_, tch):
                st = {}

                def mk(c):
                    def op():
                        if c == 0:
                            st["ps"] = psum.tile([P, 512], f32, tag="ps",
                                                 name=f"yps{dt_}_{tch}")
                        nc.tensor.matmul(
                            st["ps"],
                            lhsT=wo_sb[:, c, 128 * dt_:128 * dt_ + 128],
                            rhs=otn[c][:, 512 * tch:512 * tch + 512],
                            start=(c == 0),
                            stop=(c == 3),
                        )
                        if c == 3:
                            yst = work.tile([P, 512], bf16, tag="yst", bufs=3,
                                            name=f"yst{dt_}_{tch}")
                            nc.scalar.copy(yst, st["ps"])
                            nc.sync.dma_start(
                                out=yT[128 * dt_:128 * dt_ + 128,
                                       512 * tch:512 * tch + 512],
                                in_=yst,
                            )
                    return op

                return [mk(c) for c in range(4)]

            # Minimal upfront work (everything else becomes filler): V tiles
            # for x tch0/1, and pair 0's full Q^T/K^T. Casts on scalar (idle
            # here).
            # V(tch0) c-major: 4 psum chains open at once, so the c<4
            # matmuls run as soon as the first half-wave of x lands
            v0_ops = [v_tile_ops(tt) for tt in range(4)]
            for c in range(8):
                for ops in v0_ops:
                    ops[c]()
            for tch in range(2):
                if tch == 1:
                    for tt in range(4, 8):
                        for op in v_tile_ops(tt):
                            op()
                for op in proj_tile_ops("qt", wq_sb, 0, tch,
                                        qt_cast(0, tch, True)):
                    op()
                for op in proj_tile_ops("kt", wk_sb, 0, tch,
                                        kt_cast(0, tch, True)):
                    op()
            for tch in range(2, NQ):
                for op in proj_tile_ops("qt", wq_sb, 0, tch,
                                        qt_cast(0, tch, True)):
                    op()
                for op in proj_tile_ops("kt", wk_sb, 0, tch,
                                        kt_cast(0, tch, True)):
                    op()

            # filler queue: deferred V tiles (needed from (0,2) on), then
            # later pairs' projections; W_o chunks are appended as each
            # (3, j) block unlocks them.
            fillers = deque()
            for tt in range(8, NT):
                fillers.extend(v_tile_ops(tt))
            for a in range(1, 4):
                for tch in range(NQ):
                    fillers.extend(proj_tile_ops("qt", wq_sb, a, tch,
                                                 qt_cast(a, tch, False)))
                    fillers.extend(proj_tile_ops("kt", wk_sb, a, tch,
                                                 kt_cast(a, tch, False)))

            def pull(n):
                for _ in range(n):
                    if fillers:
                        fillers.popleft()()

            # ---- attention: head pairs, software-pipelined -------------
            # Block = (head pair a, q-block j). Both heads' scores land in
            # one 2-bank PSUM tile so a single strided exp covers them.
            # Pair 3 runs j = 1,2,3,0 so each (3,j) unlocks W_o[*,j] while
            # later blocks still provide PE cover; the short (3,0) block
            # drains W_o[*,3] at a high fill rate and only W_o[*,0] trails.
            # A block's normalize chain (z->recip->bcast->mul) is deferred
            # into the NEXT block's step loop so the vector queue serves the
            # next block's mask muls first (the AV path) and the PE keeps
            # streaming while the chain runs.
            pull_rate = {0: 3, 1: 2, 2: 2, 3: 3}
            pending_norm = []
            for a in range(4):
                for j in range(NQ):
                    av = {
                        hh: psum.tile([DK + 1, 512], f32, tag="ps",
                                      name=f"av{a}_{hh}_{j}")
                        for hh in (0, 1)
                    }
                    n_k = 4 * j + 4
                    s_tiles = {}

                    def emit_s(k0, a=a, j=j, s_tiles=s_tiles):
                        r = k0 - 4 * j
                        lo = 128 * r if r > 0 else 0
                        sp = psum2.tile([P, 2, 512], f32, tag="s2",
                                        name=f"sps{a}_{j}")
                        s_tiles[k0] = (sp, lo, r)
                        for hh in (0, 1):
                            nc.tensor.matmul(
                                sp[:, hh, lo:512],
                                lhsT=ktz[a][hh][:, P * k0:P * k0 + P],
                                rhs=qt[a][:, 512 * j + lo:512 * j + 512],
                                start=True,
                                stop=True,
                            )

                    # S runs two steps ahead of AV so the in-order PE never
                    # blocks on the exp chain or this block's av slot alloc.
                    emit_s(0)
                    emit_s(1)
                    for k0 in range(n_k):
                        s_ps, lo, r = s_tiles.pop(k0)
                        u_t = work.tile([P, 2, 512], bf16, tag="u", bufs=10,
                                        name=f"u{a}_{j}")
                        nc.scalar.activation(
                            u_t[:, :, lo:512], s_ps[:, :, lo:512], Exp,
                            scale=0.125,
                        )
                        if r >= 0:
                            nc.vector.tensor_mul(
                                u_t[:, :, lo:512],
                                u_t[:, :, lo:512],
                                mask_sb[:, r, :, lo:512],
                            )
                        rate = 1 if (a == 3 and j == 3) else pull_rate[a]
                        pull(rate)
                        if k0 == 2 and pending_norm:
                            pending_norm.pop(0)()
                        if k0 + 2 < n_k:
                            emit_s(k0 + 2)
                        for hh in (0, 1):
                            nc.tensor.matmul(
                                av[hh][:, lo:512],
                                lhsT=vt[k0][:, 2 * a + hh, :],
                                rhs=u_t[:, hh, lo:512],
                                start=(k0 == 0),
                                stop=(k0 == n_k - 1),
                            )
                    # ---- normalize: otn = av[:64] * bcast(1/Z) --------
                    def normalize(a=a, j=j, av=av):
                        for hh in (0, 1):
                            poff = 64 * hh
                            z_sb = work.tile([1, 512], f32, tag="z", bufs=2,
                                             name=f"z{a}_{j}")
                            nc.vector.tensor_copy(z_sb, av[hh][DK:DK + 1, :])
                            rz = work.tile([1, 512], f32, tag="rz", bufs=2,
                                           name=f"rz{a}_{j}")
                            nc.vector.reciprocal_approx_fast(rz, z_sb)
                            bc = work.tile([64, 512], f32, tag="bc", bufs=2,
                                           name=f"bc{a}_{j}")
                            nc.gpsimd.partition_broadcast(bc, rz)
                            nc.vector.tensor_mul(
                                otn[a][poff:poff + 64, 512 * j:512 * j + 512],
                                av[hh][0:DK, :],
                                bc,
                            )
                        if a == 3:
                            for dt_ in range(8):
                                fillers.extend(wo_tile_ops(dt_, j))

                    if a < 3 and j == 3:
                        # defer pair-boundary normalize into (a+1, 0)'s step
                        # loop: the vector queue serves the next pair's mask
                        # muls (AV critical path) before this chain
                        pending_norm.append(normalize)
                    else:
                        if a == 3 and j == 3:
                            # reserved fillers bridge the last normalize
                            # chain before W_o[*,3] unlocks
                            pull(16)
                        normalize()

            # ---- drain: whatever W_o work wasn't woven in --------------
            pull(len(fillers))

    nc.finalize()
    return nc


def _get_nc():
    if "nc" not in _CACHE:
        _CACHE["nc"] = _build()
    return _CACHE["nc"]


def kernel(x, W_q, W_k, W_v, W_o):
    import ml_dtypes
    from concourse.bass_utils import run_bass_kernel_spmd

    bf16 = ml_dtypes.bfloat16
    x = np.asarray(x, dtype=np.float32)
    W_q = np.asarray(W_q, dtype=np.float32)
    W_k = np.asarray(W_k, dtype=np.float32)
    W_v = np.asarray(W_v, dtype=np.float32)
    W_o = np.asarray(W_o, dtype=np.float32)

    kk = np.arange(P)[:, None]
    qq = np.arange(512)[None, :]
    mask = np.concatenate(
        [np.tile(qq >= kk + 128 * r, (1, 2)) for r in range(4)], axis=1
    ).astype(bf16)

    in_maps = []
    for c in range(NCORES):
        b, g = c // 2, c % 2
        rows = slice(HD * g, HD * g + HD)
        in_maps.append(
            {
                "xT": np.ascontiguousarray(x[b].T).astype(bf16),
                "wqT": np.ascontiguousarray(W_q[rows, :].T).astype(bf16),
                "wkT": np.ascontiguousarray(W_k[rows, :].T).astype(bf16),
                "wvT": np.ascontiguousarray(W_v[rows, :].T).astype(bf16),
                "woT": np.ascontiguousarray(W_o[:, rows].T).astype(bf16),
                "mask": mask,
            }
        )

    res = run_bass_kernel_spmd(_get_nc(), in_maps, list(range(NCORES)))
    y = np.zeros((B, T, D), np.float32)
    for c in range(NCORES):
        y[c // 2] += res.results[c]["yT"].T.astype(np.float32)
    return y


# revision 31
# speedup vs baseline: 1.0455x; 1.0276x over previous
"""Multi-head causal attention on 8 TRN2 NeuronCores.

Sharding: core c -> (batch b = c//2, head-group g = c%2). Each core computes
Q/K/V projections for its 8 heads (512 of the 1024 channels), causal
attention, and the row-parallel W_o partial product; the host sums the two
partials per batch (the "all-reduce").

Device layouts (per core):
  xT   (1024, 2048) bf16   x[b] transposed (channels on partitions)
  wqT  (1024, 512)  bf16   W_q[rows g].T  -> lhsT for QT = Wq_g @ xT
  wkT  (1024, 512)  bf16   same for K
  wvT  (1024, 512)  bf16   rhs for natural-layout V = x @ Wv_g.T
  woT  (512, 1024)  bf16   W_o[:, cols g].T -> lhsT for yT = Wo_g @ O^T
  mask (128, 2048)  bf16   4 diagonal-block masks (128x512 each)
  yT   (1024, 2048) bf16   partial output, transposed (host sums in f32)

Attention per head h (d_k=64): scores are computed transposed,
S^T = K_h @ Q_h^T (k on partitions, q on free axis), exp on the scalar
engine (no max subtraction: |scores/8| < ~6 at these scales), multiplicative
0/1 mask on diagonal blocks only, and P^T is consumed directly as the moving
operand of out^T = [V_h | 1]^T @ P^T, whose row 64 accumulates the softmax
denominators Z. Diagonal-crossing blocks are computed only on their valid
column range.

The PE executes matmuls strictly serially (no row-group concurrency), so
wall clock ~= total PE stream cycles + stalls. The schedule therefore aims
at (a) DMA priority order so the PE starts early and never waits on weights,
(b) "filler" matmuls (deferred V tiles, later pairs' Q^T/K^T projections,
unlocked W_o chunks) woven into the attention S->exp->AV latency chain so
the PE never idles (idle gaps also re-throttle the PE clock 2.4->1.2 GHz),
(c) off-critical engines: upfront psum->sbuf casts and y staging copies on
the scalar engine's idle phases, mask/normalize on vector, broadcasts on
gpsimd.
"""

from collections import deque

import numpy as np

B, T, D = 4, 2048, 1024
NH, DK = 16, 64
NCORES = 8
HPC = NH // 2            # heads per core
HD = HPC * DK            # 512 head-dim channels per core
P = 128                  # partitions
NT = T // P              # 16 k-tiles
NQ = T // 512            # 4 q-blocks

_CACHE = {}


def _build():
    import concourse.mybir as mybir
    import concourse.tile as tile
    from concourse import bacc
    from concourse.tile import add_dep_helper

    f32, bf16 = mybir.dt.float32, mybir.dt.bfloat16
    Exp = mybir.ActivationFunctionType.Exp

    nc = bacc.Bacc(None, target_bir_lowering=False, debug=False)
    xT = nc.dram_tensor("xT", [D, T], bf16, kind="ExternalInput")
    wqT = nc.dram_tensor("wqT", [D, HD], bf16, kind="ExternalInput")
    wkT = nc.dram_tensor("wkT", [D, HD], bf16, kind="ExternalInput")
    wvT = nc.dram_tensor("wvT", [D, HD], bf16, kind="ExternalInput")
    woT = nc.dram_tensor("woT", [HD, D], bf16, kind="ExternalInput")
    mask = nc.dram_tensor("mask", [P, 4 * 1024], bf16, kind="ExternalInput")
    yT = nc.dram_tensor("yT", [D, T], bf16, kind="ExternalOutput")

    with tile.TileContext(nc) as tc:
        with (
            tc.tile_pool(name="persist", bufs=1) as persist,
            tc.tile_pool(name="work", bufs=6) as work,
            tc.tile_pool(name="psum", bufs=4, space="PSUM") as psum,
            tc.tile_pool(name="psum2", bufs=2, space="PSUM") as psum2,
        ):
            # ---- persistent tiles --------------------------------------
            xtc = [
                [persist.tile([P, 512], bf16, tag=f"x{c}_{t}", name=f"x{c}_{t}")
                 for t in range(NQ)]
                for c in range(8)
            ]
            wq_sb = persist.tile([P, 8, HD], bf16, tag="wq")
            wk_sb = persist.tile([P, 8, HD], bf16, tag="wk")
            wv_sb = persist.tile([P, 8, HD], bf16, tag="wv")
            wo_sb = persist.tile([P, 4, D], bf16, tag="wo")
            mask_sb = persist.tile([P, 4, 2, 512], bf16, tag="mask")
            qt = [persist.tile([P, T], bf16, tag=f"qt{a}", name=f"qt{a}")
                  for a in range(4)]
            # K^T stored zero-padded per head: ktz[a][hh] has head hh's 64
            # d_k rows in their partition range and ZEROS in the other 64,
            # so S matmuls use a full K=128 lhsT (enables FWL, which hides
            # LDWEIGHTS; K=64 matmuls pay ~100ns exposed load each).
            ktz = [[persist.tile([P, T], bf16, tag=f"kt{a}_{hh}",
                                 name=f"kt{a}_{hh}")
                    for hh in (0, 1)]
                   for a in range(4)]
            vt = [persist.tile([P, HPC, DK + 1], bf16, tag=f"v{tt}", name=f"v{tt}")
                  for tt in range(NT)]
            otn = [persist.tile([P, T], bf16, tag=f"otn{i}", name=f"otn{i}")
                   for i in range(4)]

            # ---- input DMAs -------------------------------------------
            # DMA pages round-robin across all queues, so anything enqueued
            # early steals bandwidth from everything else. Enforce priority
            # WAVES with deps (a dep delays the enqueue): wave0 = wv + x
            # tch0 (first V matmuls), wave1 = wq/wk + x tch1 (pair-0
            # projections), wave2 = mask + x tch2/3 (rest of upfront),
            # wave3 = wo (needed only at pair 3).
            nc.sync.dma_start(out=wv_sb, in_=wvT.rearrange("(co p) d -> p co d", p=P))
            xT_r = xT.rearrange("(co p) t -> co p t", p=P)
            xdma = {}
            wdma = {}

            def wdma_start(nm, dst, srcp, gates):
                if srcp is None:
                    ins = nc.sync.dma_start(
                        out=mask_sb,
                        in_=mask.rearrange("p (r g q) -> p r g q", g=2, q=512),
                    )
                else:
                    ins = nc.sync.dma_start(
                        out=dst, in_=srcp.rearrange("(co p) d -> p co d", p=P)
                    )
                for g in gates:
                    add_dep_helper(ins.ins, g, sync=True,
                                   reason="DMA priority wave")
                wdma[nm] = ins.ins

            for c in range(8):
                ins = nc.sync.dma_start(out=xtc[c][0], in_=xT_r[c][:, 0:512])
                xdma[c, 0] = ins.ins
            wdma_start("wq", wq_sb, wqT, [xdma[5, 0]])
            wdma_start("wk", wk_sb, wkT, [xdma[6, 0]])
            for c in range(8):
                ins = nc.sync.dma_start(out=xtc[c][1],
                                        in_=xT_r[c][:, 512:1024])
                add_dep_helper(ins.ins, xdma[c, 0], sync=True,
                               reason="DMA priority wave")
                xdma[c, 1] = ins.ins
            wdma_start("mask", mask_sb, None, [wdma["wk"]])
            for tch in (2, 3):
                for c in range(8):
                    ins = nc.sync.dma_start(
                        out=xtc[c][tch],
                        in_=xT_r[c][:, 512 * tch:512 * tch + 512],
                    )
                    add_dep_helper(ins.ins, xdma[c, tch - 1], sync=True,
                                   reason="DMA priority wave")
                    if tch == 2:
                        add_dep_helper(ins.ins, wdma["wq"], sync=True,
                                       reason="DMA priority wave")
                    xdma[c, tch] = ins.ins
            wdma_start("wo", wo_sb, woT, [wdma["mask"]])
            for tt in range(NT):
                nc.vector.memset(vt[tt][:, :, DK:DK + 1], 1.0)
            for a in range(4):
                nc.vector.memset(ktz[a][0][64:128, :], 0.0)
                nc.vector.memset(ktz[a][1][0:64, :], 0.0)

            # ---- op builders (each closure emits one PE matmul) --------
            def v_tile_ops(tt):
                st = {}

                def mk(c):
                    def op():
                        if c == 0:
                            st["ps"] = psum.tile([P, HD], f32, tag="ps",
                                                 name=f"vps{tt}")
                        nc.tensor.matmul(
                            st["ps"],
                            lhsT=xtc[c][tt // 4][:, P * (tt % 4):P * (tt % 4) + P],
                            rhs=wv_sb[:, c, :],
                            start=(c == 0),
                            stop=(c == 7),
                        )
                        if c == 7:
                            nc.scalar.copy(
                                vt[tt][:, :, 0:DK],
                                st["ps"].rearrange("p (h e) -> p h e", e=DK),
                            )
                    return op

                return [mk(c) for c in range(8)]

            def proj_tile_ops(nm, w_sb, a, tch, cast):
                st = {}

                def mk(c):
                    def op():
                        if c == 0:
                            st["ps"] = psum.tile([P, 512], f32, tag="ps",
                                                 name=f"{nm}ps{a}_{tch}")
                        nc.tensor.matmul(
                            st["ps"],
                            lhsT=w_sb[:, c, 128 * a:128 * a + 128],
                            rhs=xtc[c][tch],
                            start=(c == 0),
                            stop=(c == 7),
                        )
                        if c == 7:
                            cast(st["ps"])
                    return op

                return [mk(c) for c in range(8)]

            def qt_cast(a, tch, on_scalar):
                def cast(ps):
                    dst = qt[a][:, 512 * tch:512 * tch + 512]
                    if on_scalar:
                        nc.scalar.copy(dst, ps)
                    else:
                        nc.vector.tensor_copy(dst, ps)
                return cast

            def kt_cast(a, tch, on_scalar):
                def cast(ps):
                    sl = slice(512 * tch, 512 * tch + 512)
                    for hh in (0, 1):
                        rows = slice(64 * hh, 64 * hh + 64)
                        dst = ktz[a][hh][rows, sl]
                        if on_scalar:
                            nc.scalar.copy(dst, ps[rows, :])
                        else:
                            nc.vector.tensor_copy(dst, ps[rows, :])
                return cast

            def wo_tile_ops(dt_, tch):
                st = {}

                def mk(c):
                    def op():
                        if c == 0:
                            st["ps"] = psum.tile([P, 512], f32, tag="ps",
                                                 name=f"yps{dt_}_{tch}")
                        nc.tensor.matmul(
                            st["ps"],
                            lhsT=wo_sb[:, c, 128 * dt_:128 * dt_ + 128],
                            rhs=otn[c][:, 512 * tch:512 * tch + 512],
                            start=(c == 0),
                            stop=(c == 3),
                        )
                        if c == 3:
                            yst = work.tile([P, 512], bf16, tag="yst", bufs=3,
                                            name=f"yst{dt_}_{tch}")
                            nc.scalar.copy(yst, st["ps"])
                            nc.sync.dma_start(
                                out=yT[128 * dt_:128 * dt_ + 128,
                                       512 * tch:512 * tch + 512],
                                in_=yst,
                            )
                    return op

                return [mk(c) for c in range(4)]

            # Minimal upfront work (everything else becomes filler): V tiles
            # for x tch0/1, and pair 0's full Q^T/K^T. Casts on scalar (idle
            # here).
            for tch in range(2):
                for tt in range(4 * tch, 4 * tch + 4):
                    for op in v_tile_ops(tt):
                        op()
                for op in proj_tile_ops("qt", wq_sb, 0, tch,
                                        qt_cast(0, tch, True)):
                    op()
                for op in proj_tile_ops("kt", wk_sb, 0, tch,
                                        kt_cast(0, tch, True)):
                    op()
            for tch in range(2, NQ):
                for op in proj_tile_ops("qt", wq_sb, 0, tch,
                                        qt_cast(0, tch, True)):
                    op()
                for op in proj_tile_ops("kt", wk_sb, 0, tch,
                                        kt_cast(0, tch, True)):
                    op()

            # filler queue: deferred V tiles (needed from (0,2) on), then
            # later pairs' projections; W_o chunks are appended as each
            # (3, j) block unlocks them.
            fillers = deque()
            for tt in range(8, NT):
                fillers.extend(v_tile_ops(tt))
            for a in range(1, 4):
                for tch in range(NQ):
                    fillers.extend(proj_tile_ops("qt", wq_sb, a, tch,
                                                 qt_cast(a, tch, False)))
                    fillers.extend(proj_tile_ops("kt", wk_sb, a, tch,
                                                 kt_cast(a, tch, False)))

            def pull(n):
                for _ in range(n):
                    if fillers:
                        fillers.popleft()()

            # ---- attention: head pairs, software-pipelined -------------
            # Block = (head pair a, q-block j). Both heads' scores land in
            # one 2-bank PSUM tile so a single strided exp covers them.
            # Pair 3 runs j = 1,2,3,0 so each (3,j) unlocks W_o[*,j] while
            # later blocks still provide PE cover; the short (3,0) block
            # drains W_o[*,3] at a high fill rate and only W_o[*,0] trails.
            # A block's normalize chain (z->recip->bcast->mul) is deferred
            # into the NEXT block's step loop so the vector queue serves the
            # next block's mask muls first (the AV path) and the PE keeps
            # streaming while the chain runs.
            pull_rate = {0: 3, 1: 2, 2: 2, 3: 3}
            pending_norm = []
            for a in range(4):
                for j in range(NQ):
                    av = {
                        hh: psum.tile([DK + 1, 512], f32, tag="ps",
                                      name=f"av{a}_{hh}_{j}")
                        for hh in (0, 1)
                    }
                    n_k = 4 * j + 4
                    s_tiles = {}

                    def emit_s(k0, a=a, j=j, s_tiles=s_tiles):
                        r = k0 - 4 * j
                        lo = 128 * r if r > 0 else 0
                        sp = psum2.tile([P, 2, 512], f32, tag="s2",
                                        name=f"sps{a}_{j}")
                        s_tiles[k0] = (sp, lo, r)
                        for hh in (0, 1):
                            nc.tensor.matmul(
                                sp[:, hh, lo:512],
                                lhsT=ktz[a][hh][:, P * k0:P * k0 + P],
                                rhs=qt[a][:, 512 * j + lo:512 * j + 512],
                                start=True,
                                stop=True,
                            )

                    # S runs two steps ahead of AV so the in-order PE never
                    # blocks on the exp chain or this block's av slot alloc.
                    emit_s(0)
                    emit_s(1)
                    for k0 in range(n_k):
                        s_ps, lo, r = s_tiles.pop(k0)
                        u_t = work.tile([P, 2, 512], bf16, tag="u", bufs=10,
                                        name=f"u{a}_{j}")
                        nc.scalar.activation(
                            u_t[:, :, lo:512], s_ps[:, :, lo:512], Exp,
                            scale=0.125,
                        )
                        if r >= 0:
                            nc.vector.tensor_mul(
                                u_t[:, :, lo:512],
                                u_t[:, :, lo:512],
                                mask_sb[:, r, :, lo:512],
                            )
                        rate = 1 if (a == 3 and j == 3) else pull_rate[a]
                        pull(rate)
                        if k0 == 2 and pending_norm:
                            pending_norm.pop(0)()
                        if k0 + 2 < n_k:
                            emit_s(k0 + 2)
                        for hh in (0, 1):
                            nc.tensor.matmul(
                                av[hh][:, lo:512],
                                lhsT=vt[k0][:, 2 * a + hh, :],
                                rhs=u_t[:, hh, lo:512],
                                start=(k0 == 0),
                                stop=(k0 == n_k - 1),
                            )
                    # ---- normalize: otn = av[:64] * bcast(1/Z) --------
                    def normalize(a=a, j=j, av=av):
                        for hh in (0, 1):
                            poff = 64 * hh
                            z_sb = work.tile([1, 512], f32, tag="z", bufs=2,
                                             name=f"z{a}_{j}")
                            nc.vector.tensor_copy(z_sb, av[hh][DK:DK + 1, :])
                            rz = work.tile([1, 512], f32, tag="rz", bufs=2,
                                           name=f"rz{a}_{j}")
                            nc.vector.reciprocal_approx_fast(rz, z_sb)
                            bc = work.tile([64, 512], f32, tag="bc", bufs=2,
                                           name=f"bc{a}_{j}")
                            nc.gpsimd.partition_broadcast(bc, rz)
                            nc.vector.tensor_mul(
                                otn[a][poff:poff + 64, 512 * j:512 * j + 512],
                                av[hh][0:DK, :],
                                bc,
                            )
                        if a == 3:
                            for dt_ in range(8):
                                fillers.extend(wo_tile_ops(dt_, j))

                    if a < 3 and j == 3:
                        # defer pair-boundary normalize into (a+1, 0)'s step
                        # loop: the vector queue serves the next pair's mask
                        # muls (AV critical path) before this chain
                        pending_norm.append(normalize)
                    else:
                        if a == 3 and j == 3:
                            # reserved fillers bridge the last normalize
                            # chain before W_o[*,3] unlocks
                            pull(16)
                        normalize()

            # ---- drain: whatever W_o work wasn't woven in --------------
            pull(len(fillers))

    nc.finalize()
    return nc


def _get_nc():
    if "nc" not in _CACHE:
        _CACHE["nc"] = _build()
    return _CACHE["nc"]


def kernel(x, W_q, W_k, W_v, W_o):
    import ml_dtypes
    from concourse.bass_utils import run_bass_kernel_spmd

    bf16 = ml_dtypes.bfloat16
    x = np.asarray(x, dtype=np.float32)
    W_q = np.asarray(W_q, dtype=np.float32)
    W_k = np.asarray(W_k, dtype=np.float32)
    W_v = np.asarray(W_v, dtype=np.float32)
    W_o = np.asarray(W_o, dtype=np.float32)

    kk = np.arange(P)[:, None]
    qq = np.arange(512)[None, :]
    mask = np.concatenate(
        [np.tile(qq >= kk + 128 * r, (1, 2)) for r in range(4)], axis=1
    ).astype(bf16)

    in_maps = []
    for c in range(NCORES):
        b, g = c // 2, c % 2
        rows = slice(HD * g, HD * g + HD)
        in_maps.append(
            {
                "xT": np.ascontiguousarray(x[b].T).astype(bf16),
                "wqT": np.ascontiguousarray(W_q[rows, :].T).astype(bf16),
                "wkT": np.ascontiguousarray(W_k[rows, :].T).astype(bf16),
                "wvT": np.ascontiguousarray(W_v[rows, :].T).astype(bf16),
                "woT": np.ascontiguousarray(W_o[:, rows].T).astype(bf16),
                "mask": mask,
            }
        )

    res = run_bass_kernel_spmd(_get_nc(), in_maps, list(range(NCORES)))
    y = np.zeros((B, T, D), np.float32)
    for c in range(NCORES):
        y[c // 2] += res.results[c]["yT"].T.astype(np.float32)
    return y
